# revision 8
# baseline (speedup 1.0000x reference)
"""ODE-RNN encoder (GRU-ODE scan) Trainium2 Bass kernel.

Strategy (data-parallel over trajectories):
  - 4096 trajectories sharded 512/core over 8 NeuronCores; all weights
    replicated. The T=128 time scan runs locally per core, no cross-core
    communication. Host gathers the per-core z0 outputs at the end.
  - On-chip layout is feature-on-partition, batch-on-free-dim. Each core's
    512-batch is split into 2 dephased chunks of 256 so the serial
    per-step dependency chain of one chunk hides under engine work of the
    other.
  - Matmuls run as float32r (full PE rate at N>=256), weights stationary in
    SBUF for all 128 steps. PSUM accumulation implements the ODE Euler step
    (y + dt*mlp via identity-matmul accumulate, dt folded into a scaled
    copy of ode_w2 per distinct dt value).
  - Gate algebra is restructured to minimize vector work:
      v = 1-u = sigmoid(-mlp_u)   (negated+duplicated ug_w2 -> [128] rows)
      r2 = sigmoid(mlp_r)         (duplicated rg_w2 -> [128] rows)
      state' = S + (m*v) * (ns' - S)   with S = [y_ode; s] stacked [128,B]
    The trailing abs of the reference is a provable no-op (s stays >= 0).
  - ACT ops fused pairwise (ug|rg tanh, v|r sigmoid share one PSUM bank);
    mask multiplies run on GPSIMD; mask duplication via SBUF->SBUF DMA.

kernel(**inputs) takes the full unsharded numpy inputs (as produced by the
reference setup) and returns (z0_mu, z0_std), each (1, 4096, 64) float32.
"""

import os
import sys

import numpy as np

N_TRAJ = 4096
T = 128
LAT = 64
NDATA = 64
INP = 2 * NDATA
NGRU = 100
NODE = 100
TZ = 100
NCORES = 8
B = N_TRAJ // NCORES          # 512 per core
CH = 2                        # chunks per core
BC = B // CH                  # 256 batch per chunk

_cache = {}


def _build(n_dt, dt_idx, use_bias):
    import concourse.bass as bass
    import concourse.tile as tile
    from concourse import bacc, mybir

    f32 = mybir.dt.float32
    f32r = mybir.dt.float32r
    ACT = mybir.ActivationFunctionType

    nc = bacc.Bacc("TRN2", target_bir_lowering=False, debug=False,
                   num_devices=NCORES)

    # ---- DRAM I/O ----
    xT_d = nc.dram_tensor("xT", [T, INP, B], f32r, kind="ExternalInput")
    wug1_d = nc.dram_tensor("wug1", [2 * LAT + INP, NGRU], f32r, kind="ExternalInput")
    wrg1_d = nc.dram_tensor("wrg1", [2 * LAT + INP, NGRU], f32r, kind="ExternalInput")
    wns1_d = nc.dram_tensor("wns1", [2 * LAT + INP, NGRU], f32r, kind="ExternalInput")
    wug2_d = nc.dram_tensor("wug2nd", [NGRU, 2 * LAT], f32r, kind="ExternalInput")
    wrg2_d = nc.dram_tensor("wrg2d", [NGRU, 2 * LAT], f32r, kind="ExternalInput")
    wns2_d = nc.dram_tensor("wns2", [NGRU, 2 * LAT], f32r, kind="ExternalInput")
    wode1_d = nc.dram_tensor("wode1", [LAT, NODE], f32r, kind="ExternalInput")
    wode2_d = nc.dram_tensor("wode2s", [n_dt, NODE, LAT], f32r, kind="ExternalInput")
    eye_d = nc.dram_tensor("eye64", [LAT, LAT], f32r, kind="ExternalInput")
    wtz1_d = nc.dram_tensor("wtz1", [2 * LAT, TZ], f32r, kind="ExternalInput")
    wtz2_d = nc.dram_tensor("wtz2", [TZ, 2 * LAT], f32r, kind="ExternalInput")
    if use_bias:
        bode1_d = nc.dram_tensor("bode1", [NODE, 1], f32, kind="ExternalInput")
        bns1_d = nc.dram_tensor("bns1", [NGRU, 1], f32, kind="ExternalInput")
        bns2b_d = nc.dram_tensor("bns2b", [LAT, 1], f32, kind="ExternalInput")
        btz1_d = nc.dram_tensor("btz1", [TZ, 1], f32, kind="ExternalInput")
        btz2t_d = nc.dram_tensor("btz2t", [LAT, 1], f32, kind="ExternalInput")
        btz2b_d = nc.dram_tensor("btz2b", [LAT, 1], f32, kind="ExternalInput")
        # row-vector biases (K=1 matmul accumulate): [1, M]
        bug1_d = nc.dram_tensor("bug1r", [1, NGRU], f32r, kind="ExternalInput")
        brg1_d = nc.dram_tensor("brg1r", [1, NGRU], f32r, kind="ExternalInput")
        bug2_d = nc.dram_tensor("bug2ndr", [1, 2 * LAT], f32r, kind="ExternalInput")
        brg2_d = nc.dram_tensor("brg2dr", [1, 2 * LAT], f32r, kind="ExternalInput")
        bns2t_d = nc.dram_tensor("bns2tr", [1, LAT], f32r, kind="ExternalInput")
        bode2_d = nc.dram_tensor("bode2r", [n_dt, 1, LAT], f32r, kind="ExternalInput")
        ones_d = nc.dram_tensor("ones1", [1, BC], f32r, kind="ExternalInput")
    zeros_d = nc.dram_tensor("zeros0", [2 * LAT, B], f32r, kind="ExternalInput")
    zout_d = nc.dram_tensor("zout", [2 * LAT, B], f32, kind="ExternalOutput")

    def r(ap):
        return ap.bitcast(f32r)

    def c32(ap):
        return ap.bitcast(f32)

    with tile.TileContext(nc) as tc:
        with (
            tc.tile_pool(name="const", bufs=1) as cpool,
            tc.tile_pool(name="state", bufs=1) as spool,
            tc.tile_pool(name="xin", bufs=3) as xpool,
            tc.tile_pool(name="mdup", bufs=2) as mpool,
            tc.tile_pool(name="tmp0", bufs=2) as tpool0,
            tc.tile_pool(name="tmp1", bufs=2) as tpool1,
            tc.tile_pool(name="psA0", bufs=1, space="PSUM") as psA0,
            tc.tile_pool(name="psB0", bufs=1, space="PSUM") as psB0,
            tc.tile_pool(name="g1p0", bufs=1, space="PSUM") as g1p0,
            tc.tile_pool(name="g2p0", bufs=1, space="PSUM") as g2p0,
            tc.tile_pool(name="psA1", bufs=1, space="PSUM") as psA1,
            tc.tile_pool(name="psB1", bufs=1, space="PSUM") as psB1,
            tc.tile_pool(name="g1p1", bufs=1, space="PSUM") as g1p1,
            tc.tile_pool(name="g2p1", bufs=1, space="PSUM") as g2p1,
        ):
            tpool = [tpool0, tpool1]
            psA = [psA0, psA1]
            psB = [psB0, psB1]
            g1p = [g1p0, g1p1]
            g2p = [g2p0, g2p1]

            # ---- load constants ----
            def cload(shape, src_ap, tag, dt_=None):
                t = cpool.tile(shape, dt_ or f32r, tag=tag, name=tag)
                nc.sync.dma_start(t[:, :], src_ap)
                return t

            wug1a = cload([INP, NGRU], wug1_d[0:INP, :], "wug1a")
            wug1b = cload([INP, NGRU], wug1_d[INP:2 * LAT + INP, :], "wug1b")
            wrg1a = cload([INP, NGRU], wrg1_d[0:INP, :], "wrg1a")
            wrg1b = cload([INP, NGRU], wrg1_d[INP:2 * LAT + INP, :], "wrg1b")
            wns1a = cload([INP, NGRU], wns1_d[0:INP, :], "wns1a")
            wns1b = cload([INP, NGRU], wns1_d[INP:2 * LAT + INP, :], "wns1b")
            # note: rows 0:128 of w*1 multiply [y;s] (=128 rows), rows 128:256
            # multiply x (=128 rows); INP == 2*LAT == 128 here.
            wug2 = cload([NGRU, 2 * LAT], wug2_d[:, :], "wug2")
            wrg2 = cload([NGRU, 2 * LAT], wrg2_d[:, :], "wrg2")
            wns2 = cload([NGRU, 2 * LAT], wns2_d[:, :], "wns2")
            wode1 = cload([LAT, NODE], wode1_d[:, :], "wode1")
            eye64 = cload([LAT, LAT], eye_d[:, :], "eye64")
            wtz1 = cload([2 * LAT, TZ], wtz1_d[:, :], "wtz1")
            wtz2 = cload([TZ, 2 * LAT], wtz2_d[:, :], "wtz2")
            wode2 = [cload([NODE, LAT], wode2_d[i], f"wode2_{i}")
                     for i in range(n_dt)]
            if use_bias:
                bode1 = cload([NODE, 1], bode1_d[:, :], "bode1", f32)
                bns1 = cload([NGRU, 1], bns1_d[:, :], "bns1", f32)
                btz1 = cload([TZ, 1], btz1_d[:, :], "btz1", f32)
                btz2t = cload([LAT, 1], btz2t_d[:, :], "btz2t", f32)
                # biases applied on partitions 64:128 must live there too
                bns2b = cpool.tile([2 * LAT, 1], f32, tag="bns2b", name="bns2b")
                nc.sync.dma_start(bns2b[LAT:2 * LAT, :], bns2b_d[:, :])
                btz2b = cpool.tile([2 * LAT, 1], f32, tag="btz2b", name="btz2b")
                nc.sync.dma_start(btz2b[LAT:2 * LAT, :], btz2b_d[:, :])
                bug1r = cload([1, NGRU], bug1_d[:, :], "bug1r")
                brg1r = cload([1, NGRU], brg1_d[:, :], "brg1r")
                bug2r = cload([1, 2 * LAT], bug2_d[:, :], "bug2r")
                brg2r = cload([1, 2 * LAT], brg2_d[:, :], "brg2r")
                bns2tr = cload([1, LAT], bns2t_d[:, :], "bns2tr")
                bode2r = [cload([1, LAT], bode2_d[i], f"bode2r_{i}")
                          for i in range(n_dt)]
                ones = cpool.tile([1, BC], f32r, tag="ones", name="ones")
                nc.sync.dma_start(ones[:, :], ones_d[:, :])

            def b_act(t):  # ACT bias operand (or 0.0 when biases disabled)
                return t[:, :] if use_bias else 0.0

            # ---- state tiles (ping-pong per chunk) ----
            S = [[spool.tile([2 * LAT, BC], f32r, tag=f"s{c}_{p}",
                             name=f"s{c}_{p}")
                  for p in range(2)] for c in range(CH)]
            for c in range(CH):
                nc.sync.dma_start(S[c][0][:, :],
                                  zeros_d[:, c * BC:(c + 1) * BC])

            # ---- the scan ----
            for t in range(T):
                xt = xpool.tile([INP, B], f32r, tag="xt")
                nc.sync.dma_start(xt[:, :], xT_d[t])
                m2 = mpool.tile([INP, B], f32r, tag="m2")
                nc.sync.dma_start(m2[0:NDATA, :], xt[NDATA:INP, :])
                nc.sync.dma_start(m2[NDATA:INP, :], xt[NDATA:INP, :])

                for c in range(CH):
                    cs = slice(c * BC, (c + 1) * BC)
                    Sc = S[c][t % 2]
                    Sn = S[c][(t + 1) % 2]
                    tp = tpool[c]
                    di = dt_idx[t]

                    # --- ODE half-step (single Euler) ---
                    ps_oh = psA[c].tile([NODE, BC], f32, tag="psA")
                    nc.tensor.matmul(ps_oh[:, :], r(wode1[:, :]),
                                     r(Sc[0:LAT, :]), start=True, stop=True)
                    h_ode = tp.tile([NODE, BC], f32r, tag="h_ode")
                    nc.scalar.activation(h_ode[:, :], ps_oh[:, :], ACT.Tanh,
                                         bias=b_act(bode1) if use_bias else 0.0)
                    ps_yo = psB[c].tile([LAT, BC], f32, tag="psB")
                    nc.tensor.matmul(ps_yo[:, :], r(wode2[di][:, :]),
                                     r(h_ode[:, :]), start=True, stop=False)
                    if use_bias:
                        nc.tensor.matmul(ps_yo[:, :], r(bode2r[di][:, :]),
                                         r(ones[:, :]), start=False, stop=False)
                    nc.tensor.matmul(ps_yo[:, :], r(eye64[:, :]),
                                     r(Sc[0:LAT, :]), start=False, stop=True)
                    # y_ode overwrites the y half of the state tile
                    nc.vector.tensor_copy(Sc[0:LAT, :], ps_yo[:, :])

                    # --- update & reset gates (fused in one PSUM bank) ---
                    g1 = g1p[c].tile([NGRU, 2 * BC], f32, tag="g1")
                    nc.tensor.matmul(g1[:, 0:BC], r(wug1a[:, :]), r(Sc[:, :]),
                                     start=True, stop=False)
                    if use_bias:
                        nc.tensor.matmul(g1[:, 0:BC], r(bug1r[:, :]),
                                         r(ones[:, :]), start=False, stop=False)
                    nc.tensor.matmul(g1[:, 0:BC], r(wug1b[:, :]), r(xt[:, cs]),
                                     start=False, stop=True)
                    nc.tensor.matmul(g1[:, BC:2 * BC], r(wrg1a[:, :]), r(Sc[:, :]),
                                     start=True, stop=False)
                    if use_bias:
                        nc.tensor.matmul(g1[:, BC:2 * BC], r(brg1r[:, :]),
                                         r(ones[:, :]), start=False, stop=False)
                    nc.tensor.matmul(g1[:, BC:2 * BC], r(wrg1b[:, :]), r(xt[:, cs]),
                                     start=False, stop=True)
                    h_g = tp.tile([NGRU, 2 * BC], f32r, tag="h_g")
                    nc.scalar.activation(h_g[:, :], g1[:, :], ACT.Tanh)

                    g2 = g2p[c].tile([2 * LAT, 2 * BC], f32, tag="g2")
                    nc.tensor.matmul(g2[:, 0:BC], r(wug2[:, :]), r(h_g[:, 0:BC]),
                                     start=True, stop=use_bias is False)
                    if use_bias:
                        nc.tensor.matmul(g2[:, 0:BC], r(bug2r[:, :]),
                                         r(ones[:, :]), start=False, stop=True)
                    nc.tensor.matmul(g2[:, BC:2 * BC], r(wrg2[:, :]),
                                     r(h_g[:, BC:2 * BC]),
                                     start=True, stop=use_bias is False)
                    if use_bias:
                        nc.tensor.matmul(g2[:, BC:2 * BC], r(brg2r[:, :]),
                                         r(ones[:, :]), start=False, stop=True)
                    vr = tp.tile([2 * LAT, 2 * BC], f32, tag="vr")
                    nc.scalar.activation(vr[:, :], g2[:, :], ACT.Sigmoid)
                    # vr[:, 0:BC] = v = 1-u (dup), vr[:, BC:2BC] = r (dup)

                    # --- candidate-state MLP ---
                    ryc = tp.tile([2 * LAT, BC], f32r, tag="ryc")
                    nc.vector.tensor_mul(ryc[:, :], vr[:, BC:2 * BC],
                                         c32(Sc[:, :]))
                    n1 = psA[c].tile([NGRU, BC], f32, tag="psA")
                    nc.tensor.matmul(n1[:, :], r(wns1a[:, :]), r(ryc[:, :]),
                                     start=True, stop=False)
                    nc.tensor.matmul(n1[:, :], r(wns1b[:, :]), r(xt[:, cs]),
                                     start=False, stop=True)
                    h_n = tp.tile([NGRU, BC], f32r, tag="h_n")
                    nc.scalar.activation(h_n[:, :], n1[:, :], ACT.Tanh,
                                         bias=b_act(bns1) if use_bias else 0.0)
                    n2 = psB[c].tile([2 * LAT, BC], f32, tag="psB")
                    nc.tensor.matmul(n2[:, :], r(wns2[:, :]), r(h_n[:, :]),
                                     start=True, stop=use_bias is False)
                    if use_bias:
                        nc.tensor.matmul(n2[0:LAT, :], r(bns2tr[:, :]),
                                         r(ones[:, :]), start=False, stop=True)

                    # --- blend: S' = S + (m*v)*(ns' - S) ---
                    nb = tp.tile([2 * LAT, BC], f32, tag="nb")
                    nc.scalar.activation(nb[LAT:2 * LAT, :], n2[LAT:2 * LAT, :],
                                         ACT.Abs,
                                         bias=bns2b[LAT:2 * LAT, :] if use_bias else 0.0)
                    q = tp.tile([2 * LAT, BC], f32, tag="q")
                    nc.vector.tensor_sub(q[0:LAT, :], n2[0:LAT, :],
                                         c32(Sc[0:LAT, :]))
                    nc.gpsimd.tensor_sub(q[LAT:2 * LAT, :], nb[LAT:2 * LAT, :],
                                         c32(Sc[LAT:2 * LAT, :]))
                    tt = tp.tile([2 * LAT, BC], f32, tag="tt")
                    nc.vector.tensor_mul(tt[:, :], vr[:, 0:BC], q[:, :])
                    gt = tp.tile([2 * LAT, BC], f32, tag="gt")
                    nc.gpsimd.tensor_mul(gt[:, :], c32(m2[:, cs]), tt[:, :])
                    nc.vector.tensor_add(Sn[:, :], c32(Sc[:, :]), gt[:, :])

            # ---- final transform z0 = mlp2([y; s]) ----
            for c in range(CH):
                cs = slice(c * BC, (c + 1) * BC)
                Sf = S[c][T % 2]
                pt1 = psA[c].tile([TZ, BC], f32, tag="psA")
                nc.tensor.matmul(pt1[:, :], r(wtz1[:, :]), r(Sf[:, :]),
                                 start=True, stop=True)
                h_t = tpool[c].tile([TZ, BC], f32r, tag="h_t")
                nc.scalar.activation(h_t[:, :], pt1[:, :], ACT.Tanh,
                                     bias=b_act(btz1) if use_bias else 0.0)
                pt2 = psB[c].tile([2 * LAT, BC], f32, tag="psB")
                nc.tensor.matmul(pt2[:, :], r(wtz2[:, :]), r(h_t[:, :]),
                                 start=True, stop=True)
                zo = tpool[c].tile([2 * LAT, BC], f32, tag="zo")
                nc.scalar.activation(zo[0:LAT, :], pt2[0:LAT, :], ACT.Copy,
                                     bias=b_act(btz2t) if use_bias else 0.0)
                nc.scalar.activation(zo[LAT:2 * LAT, :], pt2[LAT:2 * LAT, :],
                                     ACT.Abs,
                                     bias=btz2b[LAT:2 * LAT, :] if use_bias else 0.0)
                nc.sync.dma_start(zout_d[:, cs], zo[:, :])

    nc.compile()
    return nc


def _prep(inputs):
    g = lambda k: np.ascontiguousarray(np.asarray(inputs[k], dtype=np.float32))
    data = g("data")
    tps = g("tps")
    W = {k: g(k) for k in (
        "ug_w1", "ug_b1", "ug_w2", "ug_b2", "rg_w1", "rg_b1", "rg_w2", "rg_b2",
        "ns_w1", "ns_b1", "ns_w2", "ns_b2", "ode_w1", "ode_b1", "ode_w2",
        "ode_b2", "tz_w1", "tz_b1", "tz_w2", "tz_b2")}

    rev = tps[::-1]
    dts = np.concatenate([np.full((1,), -0.01, np.float32),
                          rev[1:] - rev[:-1]]).astype(np.float32)
    uniq = list(dict.fromkeys(dts.tolist()))
    dt_idx = tuple(uniq.index(d) for d in dts.tolist())
    n_dt = len(uniq)

    use_bias = any(float(np.abs(W[k]).max()) != 0.0 for k in W if "_b" in k)

    # time-reverse + transpose: [T, INP, N_TRAJ], contiguous
    xT_full = np.ascontiguousarray(data[:, ::-1, :].transpose(1, 2, 0))

    wode2s = np.stack([W["ode_w2"] * np.float32(d) for d in uniq])
    common = {
        "wug1": W["ug_w1"],
        "wrg1": W["rg_w1"],
        "wns1": W["ns_w1"],
        "wug2nd": -np.concatenate([W["ug_w2"], W["ug_w2"]], axis=1),
        "wrg2d": np.concatenate([W["rg_w2"], W["rg_w2"]], axis=1),
        "wns2": W["ns_w2"],
        "wode1": W["ode_w1"],
        "wode2s": wode2s,
        "eye64": np.eye(LAT, dtype=np.float32),
        "wtz1": W["tz_w1"],
        "wtz2": W["tz_w2"],
        "zeros0": np.zeros((2 * LAT, B), np.float32),
    }
    if use_bias:
        col = lambda v: np.ascontiguousarray(v.reshape(-1, 1))
        row = lambda v: np.ascontiguousarray(v.reshape(1, -1))
        common.update({
            "bode1": col(W["ode_b1"]),
            "bns1": col(W["ns_b1"]),
            "bns2b": col(W["ns_b2"][LAT:]),
            "btz1": col(W["tz_b1"]),
            "btz2t": col(W["tz_b2"][:LAT]),
            "btz2b": col(W["tz_b2"][LAT:]),
            "bug1r": row(W["ug_b1"]),
            "brg1r": row(W["rg_b1"]),
            "bug2ndr": row(-np.concatenate([W["ug_b2"], W["ug_b2"]])),
            "brg2dr": row(np.concatenate([W["rg_b2"], W["rg_b2"]])),
            "bns2tr": row(W["ns_b2"][:LAT]),
            "bode2r": np.stack([W["ode_b2"].reshape(1, -1) * np.float32(d)
                                for d in uniq]),
            "ones1": np.ones((1, BC), np.float32),
        })
    common = {k: np.ascontiguousarray(v.astype(np.float32))
              for k, v in common.items()}

    in_maps = []
    for c in range(NCORES):
        m = dict(common)
        m["xT"] = np.ascontiguousarray(xT_full[:, :, c * B:(c + 1) * B])
        in_maps.append(m)
    return in_maps, n_dt, dt_idx, use_bias


def _run(inputs, trace=False, trace_kwargs=None):
    from concourse.bass_utils import run_bass_kernel_spmd

    in_maps, n_dt, dt_idx, use_bias = _prep(inputs)
    key = (n_dt, dt_idx, use_bias)
    if key not in _cache:
        _cache[key] = _build(n_dt, dt_idx, use_bias)
    nc = _cache[key]

    res = run_bass_kernel_spmd(nc, in_maps, list(range(NCORES)),
                               trace=trace, **(trace_kwargs or {}))
    mu = np.empty((N_TRAJ, LAT), np.float32)
    std = np.empty((N_TRAJ, LAT), np.float32)
    for c in range(NCORES):
        z = res.results[c]["zout"]
        mu[c * B:(c + 1) * B] = z[0:LAT].T
        std[c * B:(c + 1) * B] = z[LAT:2 * LAT].T
    return (mu[None], std[None]), res


def kernel(**inputs):
    out, _ = _run(inputs, trace=False)
    return out


# revision 9
# speedup vs baseline: 1.0105x; 1.0105x over previous
"""ODE-RNN encoder (GRU-ODE scan) Trainium2 Bass kernel.

Strategy (data-parallel over trajectories):
  - 4096 trajectories sharded 512/core over 8 NeuronCores; all weights
    replicated. The T=128 time scan runs locally per core, no cross-core
    communication. Host gathers the per-core z0 outputs at the end.
  - On-chip layout is feature-on-partition, batch-on-free-dim. Each core's
    512-batch is split into 2 dephased chunks of 256 so the serial
    per-step dependency chain of one chunk hides under engine work of the
    other.
  - Matmuls run as float32r (full PE rate at N>=256), weights stationary in
    SBUF for all 128 steps. PSUM accumulation implements the ODE Euler step
    (y + dt*mlp via identity-matmul accumulate, dt folded into a scaled
    copy of ode_w2 per distinct dt value).
  - Gate algebra is restructured to minimize vector work:
      v = 1-u = sigmoid(-mlp_u)   (negated+duplicated ug_w2 -> [128] rows)
      r2 = sigmoid(mlp_r)         (duplicated rg_w2 -> [128] rows)
      state' = S + (m*v) * (ns' - S)   with S = [y_ode; s] stacked [128,B]
    The trailing abs of the reference is a provable no-op (s stays >= 0).
  - ACT ops fused pairwise (ug|rg tanh, v|r sigmoid share one PSUM bank);
    mask multiplies run on GPSIMD; mask duplication via SBUF->SBUF DMA.

kernel(**inputs) takes the full unsharded numpy inputs (as produced by the
reference setup) and returns (z0_mu, z0_std), each (1, 4096, 64) float32.
"""

import os
import sys

import numpy as np

N_TRAJ = 4096
T = 128
LAT = 64
NDATA = 64
INP = 2 * NDATA
NGRU = 100
NODE = 100
TZ = 100
NCORES = 8
B = N_TRAJ // NCORES          # 512 per core
CH = 2                        # chunks per core
BC = B // CH                  # 256 batch per chunk

_cache = {}


def _build(dts, use_bias):
    import concourse.bass as bass
    import concourse.tile as tile
    from concourse import bacc, mybir

    f32 = mybir.dt.float32
    f32r = mybir.dt.float32r
    ACT = mybir.ActivationFunctionType

    nc = bacc.Bacc("TRN2", target_bir_lowering=False, debug=False,
                   num_devices=NCORES)

    # ---- DRAM I/O ----
    xT_d = nc.dram_tensor("xT", [T, INP, B], f32r, kind="ExternalInput")
    wug1_d = nc.dram_tensor("wug1", [2 * LAT + INP, NGRU], f32r, kind="ExternalInput")
    wrg1_d = nc.dram_tensor("wrg1", [2 * LAT + INP, NGRU], f32r, kind="ExternalInput")
    wns1_d = nc.dram_tensor("wns1", [2 * LAT + INP, NGRU], f32r, kind="ExternalInput")
    wug2_d = nc.dram_tensor("wug2nd", [NGRU, 2 * LAT], f32r, kind="ExternalInput")
    wrg2_d = nc.dram_tensor("wrg2d", [NGRU, 2 * LAT], f32r, kind="ExternalInput")
    wns2_d = nc.dram_tensor("wns2", [NGRU, 2 * LAT], f32r, kind="ExternalInput")
    wode1_d = nc.dram_tensor("wode1", [LAT, NODE], f32r, kind="ExternalInput")
    wode2_d = nc.dram_tensor("wode2", [NODE, LAT], f32r, kind="ExternalInput")
    negI_d = nc.dram_tensor("negI128", [2 * LAT, 2 * LAT], f32r, kind="ExternalInput")
    wtz1_d = nc.dram_tensor("wtz1", [2 * LAT, TZ], f32r, kind="ExternalInput")
    wtz2_d = nc.dram_tensor("wtz2", [TZ, 2 * LAT], f32r, kind="ExternalInput")
    if use_bias:
        bode1_d = nc.dram_tensor("bode1", [NODE, 1], f32, kind="ExternalInput")
        bns1_d = nc.dram_tensor("bns1", [NGRU, 1], f32, kind="ExternalInput")
        bns2b_d = nc.dram_tensor("bns2b", [LAT, 1], f32, kind="ExternalInput")
        btz1_d = nc.dram_tensor("btz1", [TZ, 1], f32, kind="ExternalInput")
        btz2t_d = nc.dram_tensor("btz2t", [LAT, 1], f32, kind="ExternalInput")
        btz2b_d = nc.dram_tensor("btz2b", [LAT, 1], f32, kind="ExternalInput")
        # row-vector biases (K=1 matmul accumulate): [1, M]
        bug1_d = nc.dram_tensor("bug1r", [1, NGRU], f32r, kind="ExternalInput")
        brg1_d = nc.dram_tensor("brg1r", [1, NGRU], f32r, kind="ExternalInput")
        bug2_d = nc.dram_tensor("bug2ndr", [1, 2 * LAT], f32r, kind="ExternalInput")
        brg2_d = nc.dram_tensor("brg2dr", [1, 2 * LAT], f32r, kind="ExternalInput")
        bns2t_d = nc.dram_tensor("bns2tr", [1, LAT], f32r, kind="ExternalInput")
        bode2_d = nc.dram_tensor("bode2r", [1, LAT], f32r, kind="ExternalInput")
        ones_d = nc.dram_tensor("ones1", [1, BC], f32r, kind="ExternalInput")
    zeros_d = nc.dram_tensor("zeros0", [2 * LAT, B], f32r, kind="ExternalInput")
    zout_d = nc.dram_tensor("zout", [2 * LAT, B], f32, kind="ExternalOutput")

    def r(ap):
        return ap.bitcast(f32r)

    def c32(ap):
        return ap.bitcast(f32)

    with tile.TileContext(nc) as tc:
        with (
            tc.tile_pool(name="const", bufs=1) as cpool,
            tc.tile_pool(name="state", bufs=1) as spool,
            tc.tile_pool(name="xin", bufs=3) as xpool,
            tc.tile_pool(name="mdup", bufs=2) as mpool,
            tc.tile_pool(name="tmp0", bufs=2) as tpool0,
            tc.tile_pool(name="tmp1", bufs=2) as tpool1,
            tc.tile_pool(name="psA0", bufs=1, space="PSUM") as psA0,
            tc.tile_pool(name="psB0", bufs=1, space="PSUM") as psB0,
            tc.tile_pool(name="g1p0", bufs=1, space="PSUM") as g1p0,
            tc.tile_pool(name="g2p0", bufs=1, space="PSUM") as g2p0,
            tc.tile_pool(name="psA1", bufs=1, space="PSUM") as psA1,
            tc.tile_pool(name="psB1", bufs=1, space="PSUM") as psB1,
            tc.tile_pool(name="g1p1", bufs=1, space="PSUM") as g1p1,
            tc.tile_pool(name="g2p1", bufs=1, space="PSUM") as g2p1,
        ):
            tpool = [tpool0, tpool1]
            psA = [psA0, psA1]
            psB = [psB0, psB1]
            g1p = [g1p0, g1p1]
            g2p = [g2p0, g2p1]

            # ---- load constants ----
            def cload(shape, src_ap, tag, dt_=None):
                t = cpool.tile(shape, dt_ or f32r, tag=tag, name=tag)
                nc.sync.dma_start(t[:, :], src_ap)
                return t

            wug1a = cload([INP, NGRU], wug1_d[0:INP, :], "wug1a")
            wug1b = cload([INP, NGRU], wug1_d[INP:2 * LAT + INP, :], "wug1b")
            wrg1a = cload([INP, NGRU], wrg1_d[0:INP, :], "wrg1a")
            wrg1b = cload([INP, NGRU], wrg1_d[INP:2 * LAT + INP, :], "wrg1b")
            wns1a = cload([INP, NGRU], wns1_d[0:INP, :], "wns1a")
            wns1b = cload([INP, NGRU], wns1_d[INP:2 * LAT + INP, :], "wns1b")
            # note: rows 0:128 of w*1 multiply [y;s] (=128 rows), rows 128:256
            # multiply x (=128 rows); INP == 2*LAT == 128 here.
            wug2 = cload([NGRU, 2 * LAT], wug2_d[:, :], "wug2")
            wrg2 = cload([NGRU, 2 * LAT], wrg2_d[:, :], "wrg2")
            wns2 = cload([NGRU, 2 * LAT], wns2_d[:, :], "wns2")
            wode1 = cload([LAT, NODE], wode1_d[:, :], "wode1")
            wtz1 = cload([2 * LAT, TZ], wtz1_d[:, :], "wtz1")
            wtz2 = cload([TZ, 2 * LAT], wtz2_d[:, :], "wtz2")
            wode2 = cload([NODE, LAT], wode2_d[:, :], "wode2")
            negI = cload([2 * LAT, 2 * LAT], negI_d[:, :], "negI")
            if use_bias:
                bode1 = cload([NODE, 1], bode1_d[:, :], "bode1", f32)
                bns1 = cload([NGRU, 1], bns1_d[:, :], "bns1", f32)
                btz1 = cload([TZ, 1], btz1_d[:, :], "btz1", f32)
                btz2t = cload([LAT, 1], btz2t_d[:, :], "btz2t", f32)
                # biases applied on partitions 64:128 must live there too
                bns2b = cpool.tile([2 * LAT, 1], f32, tag="bns2b", name="bns2b")
                nc.sync.dma_start(bns2b[LAT:2 * LAT, :], bns2b_d[:, :])
                btz2b = cpool.tile([2 * LAT, 1], f32, tag="btz2b", name="btz2b")
                nc.sync.dma_start(btz2b[LAT:2 * LAT, :], btz2b_d[:, :])
                bug1r = cload([1, NGRU], bug1_d[:, :], "bug1r")
                brg1r = cload([1, NGRU], brg1_d[:, :], "brg1r")
                bug2r = cload([1, 2 * LAT], bug2_d[:, :], "bug2r")
                brg2r = cload([1, 2 * LAT], brg2_d[:, :], "brg2r")
                bns2tr = cload([1, LAT], bns2t_d[:, :], "bns2tr")
                bode2r = cload([1, LAT], bode2_d[:, :], "bode2r")
                ones = cpool.tile([1, BC], f32r, tag="ones", name="ones")
                nc.sync.dma_start(ones[:, :], ones_d[:, :])

            def b_act(t):  # ACT bias operand (or 0.0 when biases disabled)
                return t[:, :] if use_bias else 0.0

            # ---- state tiles (ping-pong per chunk) ----
            S = [[spool.tile([2 * LAT, BC], f32r, tag=f"s{c}_{p}",
                             name=f"s{c}_{p}")
                  for p in range(2)] for c in range(CH)]
            for c in range(CH):
                nc.sync.dma_start(S[c][0][:, :],
                                  zeros_d[:, c * BC:(c + 1) * BC])

            # ---- the scan ----
            for t in range(T):
                xt = xpool.tile([INP, B], f32r, tag="xt")
                nc.sync.dma_start(xt[:, :], xT_d[t])
                m2 = mpool.tile([INP, B], f32r, tag="m2")
                nc.sync.dma_start(m2[0:NDATA, :], xt[NDATA:INP, :])
                nc.sync.dma_start(m2[NDATA:INP, :], xt[NDATA:INP, :])

                for c in range(CH):
                    cs = slice(c * BC, (c + 1) * BC)
                    Sc = S[c][t % 2]
                    Sn = S[c][(t + 1) % 2]
                    tp = tpool[c]

                    # --- ODE half-step (single Euler) ---
                    ps_oh = psA[c].tile([NODE, BC], f32, tag="psA")
                    nc.tensor.matmul(ps_oh[:, :], r(wode1[:, :]),
                                     r(Sc[0:LAT, :]), start=True, stop=True)
                    h_ode = tp.tile([NODE, BC], f32r, tag="h_ode")
                    nc.scalar.activation(h_ode[:, :], ps_oh[:, :], ACT.Tanh,
                                         bias=b_act(bode1) if use_bias else 0.0)
                    ps_yo = psB[c].tile([LAT, BC], f32, tag="psB")
                    nc.tensor.matmul(ps_yo[:, :], r(wode2[:, :]),
                                     r(h_ode[:, :]), start=True,
                                     stop=not use_bias)
                    if use_bias:
                        nc.tensor.matmul(ps_yo[:, :], r(bode2r[:, :]),
                                         r(ones[:, :]), start=False, stop=True)
                    # y_ode = dt * mlp + y, overwriting the y half in place
                    nc.vector.scalar_tensor_tensor(
                        Sc[0:LAT, :], ps_yo[:, :], float(dts[t]),
                        c32(Sc[0:LAT, :]),
                        op0=mybir.AluOpType.mult, op1=mybir.AluOpType.add)

                    # --- update & reset gates (fused in one PSUM bank) ---
                    g1 = g1p[c].tile([NGRU, 2 * BC], f32, tag="g1")
                    nc.tensor.matmul(g1[:, 0:BC], r(wug1a[:, :]), r(Sc[:, :]),
                                     start=True, stop=False)
                    if use_bias:
                        nc.tensor.matmul(g1[:, 0:BC], r(bug1r[:, :]),
                                         r(ones[:, :]), start=False, stop=False)
                    nc.tensor.matmul(g1[:, 0:BC], r(wug1b[:, :]), r(xt[:, cs]),
                                     start=False, stop=True)
                    nc.tensor.matmul(g1[:, BC:2 * BC], r(wrg1a[:, :]), r(Sc[:, :]),
                                     start=True, stop=False)
                    if use_bias:
                        nc.tensor.matmul(g1[:, BC:2 * BC], r(brg1r[:, :]),
                                         r(ones[:, :]), start=False, stop=False)
                    nc.tensor.matmul(g1[:, BC:2 * BC], r(wrg1b[:, :]), r(xt[:, cs]),
                                     start=False, stop=True)
                    h_g = tp.tile([NGRU, 2 * BC], f32r, tag="h_g")
                    nc.scalar.activation(h_g[:, :], g1[:, :], ACT.Tanh)

                    g2 = g2p[c].tile([2 * LAT, 2 * BC], f32, tag="g2")
                    nc.tensor.matmul(g2[:, 0:BC], r(wug2[:, :]), r(h_g[:, 0:BC]),
                                     start=True, stop=use_bias is False)
                    if use_bias:
                        nc.tensor.matmul(g2[:, 0:BC], r(bug2r[:, :]),
                                         r(ones[:, :]), start=False, stop=True)
                    nc.tensor.matmul(g2[:, BC:2 * BC], r(wrg2[:, :]),
                                     r(h_g[:, BC:2 * BC]),
                                     start=True, stop=use_bias is False)
                    if use_bias:
                        nc.tensor.matmul(g2[:, BC:2 * BC], r(brg2r[:, :]),
                                         r(ones[:, :]), start=False, stop=True)
                    vr = tp.tile([2 * LAT, 2 * BC], f32, tag="vr")
                    nc.scalar.activation(vr[:, :], g2[:, :], ACT.Sigmoid)
                    # vr[:, 0:BC] = v = 1-u (dup), vr[:, BC:2BC] = r (dup)

                    # --- candidate-state MLP ---
                    ryc = tp.tile([2 * LAT, BC], f32r, tag="ryc")
                    nc.vector.tensor_mul(ryc[:, :], vr[:, BC:2 * BC],
                                         c32(Sc[:, :]))
                    n1 = psA[c].tile([NGRU, BC], f32, tag="psA")
                    nc.tensor.matmul(n1[:, :], r(wns1a[:, :]), r(ryc[:, :]),
                                     start=True, stop=False)
                    nc.tensor.matmul(n1[:, :], r(wns1b[:, :]), r(xt[:, cs]),
                                     start=False, stop=True)
                    h_n = tp.tile([NGRU, BC], f32r, tag="h_n")
                    nc.scalar.activation(h_n[:, :], n1[:, :], ACT.Tanh,
                                         bias=b_act(bns1) if use_bias else 0.0)
                    n2 = psB[c].tile([2 * LAT, BC], f32, tag="psB")
                    nc.tensor.matmul(n2[:, :], r(wns2[:, :]), r(h_n[:, :]),
                                     start=True, stop=False)
                    if use_bias:
                        nc.tensor.matmul(n2[0:LAT, :], r(bns2tr[:, :]),
                                         r(ones[:, :]), start=False, stop=False)

                    # --- blend: S' = S + (m*v)*(ns' - S) ---
                    # g = m*v computed early (off the ns2 critical path)
                    g = tp.tile([2 * LAT, BC], f32, tag="g")
                    nc.vector.tensor_mul(g[:, :], c32(m2[:, cs]), vr[:, 0:BC])
                    # abs of the std half, in place in PSUM; then fold the
                    # full "- S" via a -I matmul accumulate -> bank holds q
                    nc.scalar.activation(n2[LAT:2 * LAT, :], n2[LAT:2 * LAT, :],
                                         ACT.Abs,
                                         bias=bns2b[LAT:2 * LAT, :] if use_bias else 0.0)
                    nc.tensor.matmul(n2[:, :], r(negI[:, :]), r(Sc[:, :]),
                                     start=False, stop=True,
                                     skip_group_check=True)
                    gtq = tp.tile([2 * LAT, BC], f32, tag="gtq")
                    nc.vector.tensor_mul(gtq[:, :], g[:, :], n2[:, :])
                    nc.vector.tensor_add(Sn[:, :], c32(Sc[:, :]), gtq[:, :])

            # ---- final transform z0 = mlp2([y; s]) ----
            for c in range(CH):
                cs = slice(c * BC, (c + 1) * BC)
                Sf = S[c][T % 2]
                pt1 = psA[c].tile([TZ, BC], f32, tag="psA")
                nc.tensor.matmul(pt1[:, :], r(wtz1[:, :]), r(Sf[:, :]),
                                 start=True, stop=True)
                h_t = tpool[c].tile([TZ, BC], f32r, tag="h_t")
                nc.scalar.activation(h_t[:, :], pt1[:, :], ACT.Tanh,
                                     bias=b_act(btz1) if use_bias else 0.0)
                pt2 = psB[c].tile([2 * LAT, BC], f32, tag="psB")
                nc.tensor.matmul(pt2[:, :], r(wtz2[:, :]), r(h_t[:, :]),
                                 start=True, stop=True)
                zo = tpool[c].tile([2 * LAT, BC], f32, tag="zo")
                nc.scalar.activation(zo[0:LAT, :], pt2[0:LAT, :], ACT.Copy,
                                     bias=b_act(btz2t) if use_bias else 0.0)
                nc.scalar.activation(zo[LAT:2 * LAT, :], pt2[LAT:2 * LAT, :],
                                     ACT.Abs,
                                     bias=btz2b[LAT:2 * LAT, :] if use_bias else 0.0)
                nc.sync.dma_start(zout_d[:, cs], zo[:, :])

    nc.compile()
    return nc


def _prep(inputs):
    g = lambda k: np.ascontiguousarray(np.asarray(inputs[k], dtype=np.float32))
    data = g("data")
    tps = g("tps")
    W = {k: g(k) for k in (
        "ug_w1", "ug_b1", "ug_w2", "ug_b2", "rg_w1", "rg_b1", "rg_w2", "rg_b2",
        "ns_w1", "ns_b1", "ns_w2", "ns_b2", "ode_w1", "ode_b1", "ode_w2",
        "ode_b2", "tz_w1", "tz_b1", "tz_w2", "tz_b2")}

    rev = tps[::-1]
    dts = np.concatenate([np.full((1,), -0.01, np.float32),
                          rev[1:] - rev[:-1]]).astype(np.float32)
    dts = tuple(float(d) for d in dts.tolist())

    use_bias = any(float(np.abs(W[k]).max()) != 0.0 for k in W if "_b" in k)

    # time-reverse + transpose: [T, INP, N_TRAJ], contiguous
    xT_full = np.ascontiguousarray(data[:, ::-1, :].transpose(1, 2, 0))

    common = {
        "wug1": W["ug_w1"],
        "wrg1": W["rg_w1"],
        "wns1": W["ns_w1"],
        "wug2nd": -np.concatenate([W["ug_w2"], W["ug_w2"]], axis=1),
        "wrg2d": np.concatenate([W["rg_w2"], W["rg_w2"]], axis=1),
        "wns2": W["ns_w2"],
        "wode1": W["ode_w1"],
        "wode2": W["ode_w2"],
        "negI128": -np.eye(2 * LAT, dtype=np.float32),
        "wtz1": W["tz_w1"],
        "wtz2": W["tz_w2"],
        "zeros0": np.zeros((2 * LAT, B), np.float32),
    }
    if use_bias:
        col = lambda v: np.ascontiguousarray(v.reshape(-1, 1))
        row = lambda v: np.ascontiguousarray(v.reshape(1, -1))
        common.update({
            "bode1": col(W["ode_b1"]),
            "bns1": col(W["ns_b1"]),
            "bns2b": col(W["ns_b2"][LAT:]),
            "btz1": col(W["tz_b1"]),
            "btz2t": col(W["tz_b2"][:LAT]),
            "btz2b": col(W["tz_b2"][LAT:]),
            "bug1r": row(W["ug_b1"]),
            "brg1r": row(W["rg_b1"]),
            "bug2ndr": row(-np.concatenate([W["ug_b2"], W["ug_b2"]])),
            "brg2dr": row(np.concatenate([W["rg_b2"], W["rg_b2"]])),
            "bns2tr": row(W["ns_b2"][:LAT]),
            "bode2r": row(W["ode_b2"]),
            "ones1": np.ones((1, BC), np.float32),
        })
    common = {k: np.ascontiguousarray(v.astype(np.float32))
              for k, v in common.items()}

    in_maps = []
    for c in range(NCORES):
        m = dict(common)
        m["xT"] = np.ascontiguousarray(xT_full[:, :, c * B:(c + 1) * B])
        in_maps.append(m)
    return in_maps, dts, use_bias


def _run(inputs, trace=False, trace_kwargs=None):
    from concourse.bass_utils import run_bass_kernel_spmd

    in_maps, dts, use_bias = _prep(inputs)
    key = (dts, use_bias)
    if key not in _cache:
        _cache[key] = _build(dts, use_bias)
    nc = _cache[key]

    res = run_bass_kernel_spmd(nc, in_maps, list(range(NCORES)),
                               trace=trace, **(trace_kwargs or {}))
    mu = np.empty((N_TRAJ, LAT), np.float32)
    std = np.empty((N_TRAJ, LAT), np.float32)
    for c in range(NCORES):
        z = res.results[c]["zout"]
        mu[c * B:(c + 1) * B] = z[0:LAT].T
        std[c * B:(c + 1) * B] = z[LAT:2 * LAT].T
    return (mu[None], std[None]), res


def kernel(**inputs):
    out, _ = _run(inputs, trace=False)
    return out


# revision 10
# speedup vs baseline: 1.1776x; 1.1654x over previous
"""ODE-RNN encoder (GRU-ODE scan) Trainium2 Bass kernel.

Strategy (data-parallel over trajectories):
  - 4096 trajectories sharded 512/core over 8 NeuronCores; all weights
    replicated. The T=128 time scan runs locally per core, no cross-core
    communication. Host gathers the per-core z0 outputs at the end.
  - On-chip layout is feature-on-partition, batch-on-free-dim. Each core's
    512-batch is split into 2 dephased chunks of 256 so the serial
    per-step dependency chain of one chunk hides under engine work of the
    other.
  - Matmuls run as float32r (full PE rate at N>=256), weights stationary in
    SBUF for all 128 steps. PSUM accumulation implements the ODE Euler step
    (y + dt*mlp via identity-matmul accumulate, dt folded into a scaled
    copy of ode_w2 per distinct dt value).
  - Gate algebra is restructured to minimize vector work:
      v = 1-u = sigmoid(-mlp_u)   (negated+duplicated ug_w2 -> [128] rows)
      r2 = sigmoid(mlp_r)         (duplicated rg_w2 -> [128] rows)
      state' = S + (m*v) * (ns' - S)   with S = [y_ode; s] stacked [128,B]
    The trailing abs of the reference is a provable no-op (s stays >= 0).
  - ACT ops fused pairwise (ug|rg tanh, v|r sigmoid share one PSUM bank);
    mask multiplies run on GPSIMD; mask duplication via SBUF->SBUF DMA.

kernel(**inputs) takes the full unsharded numpy inputs (as produced by the
reference setup) and returns (z0_mu, z0_std), each (1, 4096, 64) float32.
"""

import os
import sys

import numpy as np

N_TRAJ = 4096
T = 128
LAT = 64
NDATA = 64
INP = 2 * NDATA
NGRU = 100
NODE = 100
TZ = 100
NCORES = 8
B = N_TRAJ // NCORES          # 512 per core
CH = 2                        # chunks per core
BC = B // CH                  # 256 batch per chunk

_cache = {}


def _build(dts, use_bias):
    import concourse.bass as bass
    import concourse.tile as tile
    from concourse import bacc, mybir

    f32 = mybir.dt.float32
    f32r = mybir.dt.float32r
    ACT = mybir.ActivationFunctionType

    nc = bacc.Bacc("TRN2", target_bir_lowering=False, debug=False,
                   num_devices=NCORES)

    # ---- DRAM I/O ----
    xT_d = nc.dram_tensor("xT", [T, INP, B], f32r, kind="ExternalInput")
    wug1_d = nc.dram_tensor("wug1", [2 * LAT + INP, NGRU], f32r, kind="ExternalInput")
    wrg1_d = nc.dram_tensor("wrg1", [2 * LAT + INP, NGRU], f32r, kind="ExternalInput")
    wns1_d = nc.dram_tensor("wns1", [2 * LAT + INP, NGRU], f32r, kind="ExternalInput")
    wug2_d = nc.dram_tensor("wug2nd", [NGRU, 2 * LAT], f32r, kind="ExternalInput")
    wrg2_d = nc.dram_tensor("wrg2d", [NGRU, 2 * LAT], f32r, kind="ExternalInput")
    wns2_d = nc.dram_tensor("wns2", [NGRU, 2 * LAT], f32r, kind="ExternalInput")
    wode1_d = nc.dram_tensor("wode1", [LAT, NODE], f32r, kind="ExternalInput")
    wode2_d = nc.dram_tensor("wode2", [NODE, LAT], f32r, kind="ExternalInput")
    negI_d = nc.dram_tensor("negI128", [2 * LAT, 2 * LAT], f32r, kind="ExternalInput")
    wtz1_d = nc.dram_tensor("wtz1", [2 * LAT, TZ], f32r, kind="ExternalInput")
    wtz2_d = nc.dram_tensor("wtz2", [TZ, 2 * LAT], f32r, kind="ExternalInput")
    if use_bias:
        bode1_d = nc.dram_tensor("bode1", [NODE, 1], f32, kind="ExternalInput")
        bns1_d = nc.dram_tensor("bns1", [NGRU, 1], f32, kind="ExternalInput")
        bns2b_d = nc.dram_tensor("bns2b", [LAT, 1], f32, kind="ExternalInput")
        btz1_d = nc.dram_tensor("btz1", [TZ, 1], f32, kind="ExternalInput")
        btz2t_d = nc.dram_tensor("btz2t", [LAT, 1], f32, kind="ExternalInput")
        btz2b_d = nc.dram_tensor("btz2b", [LAT, 1], f32, kind="ExternalInput")
        # row-vector biases (K=1 matmul accumulate): [1, M]
        bug1_d = nc.dram_tensor("bug1r", [1, NGRU], f32r, kind="ExternalInput")
        brg1_d = nc.dram_tensor("brg1r", [1, NGRU], f32r, kind="ExternalInput")
        bug2_d = nc.dram_tensor("bug2ndr", [1, 2 * LAT], f32r, kind="ExternalInput")
        brg2_d = nc.dram_tensor("brg2dr", [1, 2 * LAT], f32r, kind="ExternalInput")
        bns2t_d = nc.dram_tensor("bns2tr", [1, LAT], f32r, kind="ExternalInput")
        bode2_d = nc.dram_tensor("bode2r", [1, LAT], f32r, kind="ExternalInput")
        ones_d = nc.dram_tensor("ones1", [1, BC], f32r, kind="ExternalInput")
    zeros_d = nc.dram_tensor("zeros0", [2 * LAT, B], f32r, kind="ExternalInput")
    zout_d = nc.dram_tensor("zout", [2 * LAT, B], f32, kind="ExternalOutput")

    def r(ap):
        return ap.bitcast(f32r)

    def c32(ap):
        return ap.bitcast(f32)

    with tile.TileContext(nc) as tc:
        with (
            tc.tile_pool(name="const", bufs=1) as cpool,
            tc.tile_pool(name="state", bufs=1) as spool,
            tc.tile_pool(name="xin", bufs=3) as xpool,
            tc.tile_pool(name="mdup", bufs=2) as mpool,
            tc.tile_pool(name="tmp0", bufs=2) as tpool0,
            tc.tile_pool(name="tmp1", bufs=2) as tpool1,
            tc.tile_pool(name="psA0", bufs=1, space="PSUM") as psA0,
            tc.tile_pool(name="psB0", bufs=1, space="PSUM") as psB0,
            tc.tile_pool(name="g1p0", bufs=1, space="PSUM") as g1p0,
            tc.tile_pool(name="g2p0", bufs=1, space="PSUM") as g2p0,
            tc.tile_pool(name="psA1", bufs=1, space="PSUM") as psA1,
            tc.tile_pool(name="psB1", bufs=1, space="PSUM") as psB1,
            tc.tile_pool(name="g1p1", bufs=1, space="PSUM") as g1p1,
            tc.tile_pool(name="g2p1", bufs=1, space="PSUM") as g2p1,
        ):
            tpool = [tpool0, tpool1]
            psA = [psA0, psA1]
            psB = [psB0, psB1]
            g1p = [g1p0, g1p1]
            g2p = [g2p0, g2p1]

            # ---- load constants ----
            def cload(shape, src_ap, tag, dt_=None):
                t = cpool.tile(shape, dt_ or f32r, tag=tag, name=tag)
                nc.sync.dma_start(t[:, :], src_ap)
                return t

            wug1a = cload([INP, NGRU], wug1_d[0:INP, :], "wug1a")
            wug1b = cload([INP, NGRU], wug1_d[INP:2 * LAT + INP, :], "wug1b")
            wrg1a = cload([INP, NGRU], wrg1_d[0:INP, :], "wrg1a")
            wrg1b = cload([INP, NGRU], wrg1_d[INP:2 * LAT + INP, :], "wrg1b")
            wns1a = cload([INP, NGRU], wns1_d[0:INP, :], "wns1a")
            wns1b = cload([INP, NGRU], wns1_d[INP:2 * LAT + INP, :], "wns1b")
            # note: rows 0:128 of w*1 multiply [y;s] (=128 rows), rows 128:256
            # multiply x (=128 rows); INP == 2*LAT == 128 here.
            wug2 = cload([NGRU, 2 * LAT], wug2_d[:, :], "wug2")
            wrg2 = cload([NGRU, 2 * LAT], wrg2_d[:, :], "wrg2")
            wns2 = cload([NGRU, 2 * LAT], wns2_d[:, :], "wns2")
            wode1 = cload([LAT, NODE], wode1_d[:, :], "wode1")
            wtz1 = cload([2 * LAT, TZ], wtz1_d[:, :], "wtz1")
            wtz2 = cload([TZ, 2 * LAT], wtz2_d[:, :], "wtz2")
            wode2 = cload([NODE, LAT], wode2_d[:, :], "wode2")
            negI = cload([2 * LAT, 2 * LAT], negI_d[:, :], "negI")
            if use_bias:
                bode1 = cload([NODE, 1], bode1_d[:, :], "bode1", f32)
                bns1 = cload([NGRU, 1], bns1_d[:, :], "bns1", f32)
                btz1 = cload([TZ, 1], btz1_d[:, :], "btz1", f32)
                btz2t = cload([LAT, 1], btz2t_d[:, :], "btz2t", f32)
                # biases applied on partitions 64:128 must live there too
                bns2b = cpool.tile([2 * LAT, 1], f32, tag="bns2b", name="bns2b")
                nc.sync.dma_start(bns2b[LAT:2 * LAT, :], bns2b_d[:, :])
                btz2b = cpool.tile([2 * LAT, 1], f32, tag="btz2b", name="btz2b")
                nc.sync.dma_start(btz2b[LAT:2 * LAT, :], btz2b_d[:, :])
                bug1r = cload([1, NGRU], bug1_d[:, :], "bug1r")
                brg1r = cload([1, NGRU], brg1_d[:, :], "brg1r")
                bug2r = cload([1, 2 * LAT], bug2_d[:, :], "bug2r")
                brg2r = cload([1, 2 * LAT], brg2_d[:, :], "brg2r")
                bns2tr = cload([1, LAT], bns2t_d[:, :], "bns2tr")
                bode2r = cload([1, LAT], bode2_d[:, :], "bode2r")
                ones = cpool.tile([1, BC], f32r, tag="ones", name="ones")
                nc.sync.dma_start(ones[:, :], ones_d[:, :])

            def b_act(t):  # ACT bias operand (or 0.0 when biases disabled)
                return t[:, :] if use_bias else 0.0

            # ---- state tiles (ping-pong per chunk) ----
            S = [[spool.tile([2 * LAT, BC], f32r, tag=f"s{c}_{p}",
                             name=f"s{c}_{p}")
                  for p in range(2)] for c in range(CH)]
            for c in range(CH):
                nc.sync.dma_start(S[c][0][:, :],
                                  zeros_d[:, c * BC:(c + 1) * BC])

            # ---- the scan ----
            # Stage-interleaved emission: both chunks' ops for one stage are
            # adjacent in program order, so every engine queue alternates
            # between the two independent serial chains (software pipeline).
            for t in range(T):
                xt = xpool.tile([INP, B], f32r, tag="xt")
                nc.sync.dma_start(xt[:, :], xT_d[t])
                m2 = mpool.tile([INP, B], f32r, tag="m2")
                nc.sync.dma_start(m2[0:NDATA, :], xt[NDATA:INP, :])
                nc.sync.dma_start(m2[NDATA:INP, :], xt[NDATA:INP, :])

                st = []
                for c in range(CH):
                    cs = slice(c * BC, (c + 1) * BC)
                    Sc = S[c][t % 2]
                    Sn = S[c][(t + 1) % 2]
                    tp = tpool[c]
                    st.append(dict(cs=cs, Sc=Sc, Sn=Sn, tp=tp))

                def s_ode1(c):
                    d = st[c]
                    d['ps_oh'] = psA[c].tile([NODE, BC], f32, tag="psA",
                                             name=f"oh{c}")
                    nc.tensor.matmul(d['ps_oh'][:, :], r(wode1[:, :]),
                                     r(d['Sc'][0:LAT, :]), start=True, stop=True)

                def s_tanh_ode(c):
                    d = st[c]
                    d['h_ode'] = st[c]['tp'].tile([NODE, BC], f32r, tag="h_ode",
                                                  name=f"ho{c}")
                    nc.scalar.activation(d['h_ode'][:, :], d['ps_oh'][:, :],
                                         ACT.Tanh,
                                         bias=b_act(bode1) if use_bias else 0.0)

                def s_ode2(c):
                    d = st[c]
                    d['ps_yo'] = psB[c].tile([LAT, BC], f32, tag="psB",
                                             name=f"yo{c}")
                    nc.tensor.matmul(d['ps_yo'][:, :], r(wode2[:, :]),
                                     r(d['h_ode'][:, :]), start=True,
                                     stop=not use_bias)
                    if use_bias:
                        nc.tensor.matmul(d['ps_yo'][:, :], r(bode2r[:, :]),
                                         r(ones[:, :]), start=False, stop=True)

                def s_yode(c):
                    d = st[c]
                    nc.vector.scalar_tensor_tensor(
                        d['Sc'][0:LAT, :], d['ps_yo'][:, :], float(dts[t]),
                        c32(d['Sc'][0:LAT, :]),
                        op0=mybir.AluOpType.mult, op1=mybir.AluOpType.add)

                def s_ug1(c):
                    d = st[c]
                    d['g1'] = g1p[c].tile([NGRU, 2 * BC], f32, tag="g1",
                                          name=f"g1_{c}")
                    g1 = d['g1']
                    nc.tensor.matmul(g1[:, 0:BC], r(wug1a[:, :]),
                                     r(d['Sc'][:, :]), start=True, stop=False)
                    if use_bias:
                        nc.tensor.matmul(g1[:, 0:BC], r(bug1r[:, :]),
                                         r(ones[:, :]), start=False, stop=False)
                    nc.tensor.matmul(g1[:, 0:BC], r(wug1b[:, :]),
                                     r(xt[:, d['cs']]), start=False, stop=True)

                def s_rg1(c):
                    d = st[c]
                    g1 = d['g1']
                    nc.tensor.matmul(g1[:, BC:2 * BC], r(wrg1a[:, :]),
                                     r(d['Sc'][:, :]), start=True, stop=False)
                    if use_bias:
                        nc.tensor.matmul(g1[:, BC:2 * BC], r(brg1r[:, :]),
                                         r(ones[:, :]), start=False, stop=False)
                    nc.tensor.matmul(g1[:, BC:2 * BC], r(wrg1b[:, :]),
                                     r(xt[:, d['cs']]), start=False, stop=True)

                def s_tanh_g(c):
                    d = st[c]
                    d['h_g'] = d['tp'].tile([NGRU, 2 * BC], f32r, tag="h_g",
                                            name=f"hg{c}")
                    nc.scalar.activation(d['h_g'][:, :], d['g1'][:, :], ACT.Tanh)

                def s_g2(c):
                    d = st[c]
                    d['g2'] = g2p[c].tile([2 * LAT, 2 * BC], f32, tag="g2",
                                          name=f"g2_{c}")
                    g2, h_g = d['g2'], d['h_g']
                    nc.tensor.matmul(g2[:, 0:BC], r(wug2[:, :]),
                                     r(h_g[:, 0:BC]),
                                     start=True, stop=use_bias is False)
                    if use_bias:
                        nc.tensor.matmul(g2[:, 0:BC], r(bug2r[:, :]),
                                         r(ones[:, :]), start=False, stop=True)
                    nc.tensor.matmul(g2[:, BC:2 * BC], r(wrg2[:, :]),
                                     r(h_g[:, BC:2 * BC]),
                                     start=True, stop=use_bias is False)
                    if use_bias:
                        nc.tensor.matmul(g2[:, BC:2 * BC], r(brg2r[:, :]),
                                         r(ones[:, :]), start=False, stop=True)

                def s_sig(c):
                    d = st[c]
                    d['vr'] = d['tp'].tile([2 * LAT, 2 * BC], f32, tag="vr",
                                           name=f"vr{c}")
                    nc.scalar.activation(d['vr'][:, :], d['g2'][:, :],
                                         ACT.Sigmoid)

                def s_gm(c):
                    d = st[c]
                    d['g'] = d['tp'].tile([2 * LAT, BC], f32, tag="g",
                                          name=f"g{c}")
                    nc.vector.tensor_mul(d['g'][:, :], c32(m2[:, d['cs']]),
                                         d['vr'][:, 0:BC])

                def s_ryc(c):
                    d = st[c]
                    d['ryc'] = d['tp'].tile([2 * LAT, BC], f32r, tag="ryc",
                                            name=f"ryc{c}")
                    nc.vector.tensor_mul(d['ryc'][:, :], d['vr'][:, BC:2 * BC],
                                         c32(d['Sc'][:, :]))

                def s_ns1(c):
                    d = st[c]
                    d['n1'] = psA[c].tile([NGRU, BC], f32, tag="psA",
                                          name=f"n1_{c}")
                    nc.tensor.matmul(d['n1'][:, :], r(wns1a[:, :]),
                                     r(d['ryc'][:, :]), start=True, stop=False)
                    nc.tensor.matmul(d['n1'][:, :], r(wns1b[:, :]),
                                     r(xt[:, d['cs']]), start=False, stop=True)

                def s_tanh_ns(c):
                    d = st[c]
                    d['h_n'] = d['tp'].tile([NGRU, BC], f32r, tag="h_n",
                                            name=f"hn{c}")
                    nc.scalar.activation(d['h_n'][:, :], d['n1'][:, :], ACT.Tanh,
                                         bias=b_act(bns1) if use_bias else 0.0)

                def s_ns2(c):
                    d = st[c]
                    d['n2'] = psB[c].tile([2 * LAT, BC], f32, tag="psB",
                                          name=f"n2_{c}")
                    nc.tensor.matmul(d['n2'][:, :], r(wns2[:, :]),
                                     r(d['h_n'][:, :]), start=True, stop=False)
                    if use_bias:
                        nc.tensor.matmul(d['n2'][0:LAT, :], r(bns2tr[:, :]),
                                         r(ones[:, :]), start=False, stop=False)

                def s_abs(c):
                    d = st[c]
                    n2 = d['n2']
                    nc.scalar.activation(n2[LAT:2 * LAT, :], n2[LAT:2 * LAT, :],
                                         ACT.Abs,
                                         bias=bns2b[LAT:2 * LAT, :] if use_bias else 0.0)

                def s_negI(c):
                    d = st[c]
                    nc.tensor.matmul(d['n2'][:, :], r(negI[:, :]),
                                     r(d['Sc'][:, :]), start=False, stop=True,
                                     skip_group_check=True)

                def s_gtq(c):
                    d = st[c]
                    d['gtq'] = d['tp'].tile([2 * LAT, BC], f32, tag="gtq",
                                            name=f"gtq{c}")
                    nc.vector.tensor_mul(d['gtq'][:, :], d['g'][:, :],
                                         d['n2'][:, :])

                def s_add(c):
                    d = st[c]
                    nc.vector.tensor_add(d['Sn'][:, :], c32(d['Sc'][:, :]),
                                         d['gtq'][:, :])

                stages = [s_ode1, s_tanh_ode, s_ode2, s_yode, s_ug1, s_rg1,
                          s_tanh_g, s_g2, s_sig, s_gm, s_ryc, s_ns1, s_tanh_ns,
                          s_ns2, s_abs, s_negI, s_gtq, s_add]
                for stage in stages:
                    for c in range(CH):
                        stage(c)

            # ---- final transform z0 = mlp2([y; s]) ----
            for c in range(CH):
                cs = slice(c * BC, (c + 1) * BC)
                Sf = S[c][T % 2]
                pt1 = psA[c].tile([TZ, BC], f32, tag="psA")
                nc.tensor.matmul(pt1[:, :], r(wtz1[:, :]), r(Sf[:, :]),
                                 start=True, stop=True)
                h_t = tpool[c].tile([TZ, BC], f32r, tag="h_t")
                nc.scalar.activation(h_t[:, :], pt1[:, :], ACT.Tanh,
                                     bias=b_act(btz1) if use_bias else 0.0)
                pt2 = psB[c].tile([2 * LAT, BC], f32, tag="psB")
                nc.tensor.matmul(pt2[:, :], r(wtz2[:, :]), r(h_t[:, :]),
                                 start=True, stop=True)
                zo = tpool[c].tile([2 * LAT, BC], f32, tag="zo")
                nc.scalar.activation(zo[0:LAT, :], pt2[0:LAT, :], ACT.Copy,
                                     bias=b_act(btz2t) if use_bias else 0.0)
                nc.scalar.activation(zo[LAT:2 * LAT, :], pt2[LAT:2 * LAT, :],
                                     ACT.Abs,
                                     bias=btz2b[LAT:2 * LAT, :] if use_bias else 0.0)
                nc.sync.dma_start(zout_d[:, cs], zo[:, :])

    nc.compile()
    return nc


def _prep(inputs):
    g = lambda k: np.ascontiguousarray(np.asarray(inputs[k], dtype=np.float32))
    data = g("data")
    tps = g("tps")
    W = {k: g(k) for k in (
        "ug_w1", "ug_b1", "ug_w2", "ug_b2", "rg_w1", "rg_b1", "rg_w2", "rg_b2",
        "ns_w1", "ns_b1", "ns_w2", "ns_b2", "ode_w1", "ode_b1", "ode_w2",
        "ode_b2", "tz_w1", "tz_b1", "tz_w2", "tz_b2")}

    rev = tps[::-1]
    dts = np.concatenate([np.full((1,), -0.01, np.float32),
                          rev[1:] - rev[:-1]]).astype(np.float32)
    dts = tuple(float(d) for d in dts.tolist())

    use_bias = any(float(np.abs(W[k]).max()) != 0.0 for k in W if "_b" in k)

    # time-reverse + transpose: [T, INP, N_TRAJ], contiguous
    xT_full = np.ascontiguousarray(data[:, ::-1, :].transpose(1, 2, 0))

    common = {
        "wug1": W["ug_w1"],
        "wrg1": W["rg_w1"],
        "wns1": W["ns_w1"],
        "wug2nd": -np.concatenate([W["ug_w2"], W["ug_w2"]], axis=1),
        "wrg2d": np.concatenate([W["rg_w2"], W["rg_w2"]], axis=1),
        "wns2": W["ns_w2"],
        "wode1": W["ode_w1"],
        "wode2": W["ode_w2"],
        "negI128": -np.eye(2 * LAT, dtype=np.float32),
        "wtz1": W["tz_w1"],
        "wtz2": W["tz_w2"],
        "zeros0": np.zeros((2 * LAT, B), np.float32),
    }
    if use_bias:
        col = lambda v: np.ascontiguousarray(v.reshape(-1, 1))
        row = lambda v: np.ascontiguousarray(v.reshape(1, -1))
        common.update({
            "bode1": col(W["ode_b1"]),
            "bns1": col(W["ns_b1"]),
            "bns2b": col(W["ns_b2"][LAT:]),
            "btz1": col(W["tz_b1"]),
            "btz2t": col(W["tz_b2"][:LAT]),
            "btz2b": col(W["tz_b2"][LAT:]),
            "bug1r": row(W["ug_b1"]),
            "brg1r": row(W["rg_b1"]),
            "bug2ndr": row(-np.concatenate([W["ug_b2"], W["ug_b2"]])),
            "brg2dr": row(np.concatenate([W["rg_b2"], W["rg_b2"]])),
            "bns2tr": row(W["ns_b2"][:LAT]),
            "bode2r": row(W["ode_b2"]),
            "ones1": np.ones((1, BC), np.float32),
        })
    common = {k: np.ascontiguousarray(v.astype(np.float32))
              for k, v in common.items()}

    in_maps = []
    for c in range(NCORES):
        m = dict(common)
        m["xT"] = np.ascontiguousarray(xT_full[:, :, c * B:(c + 1) * B])
        in_maps.append(m)
    return in_maps, dts, use_bias


def _run(inputs, trace=False, trace_kwargs=None):
    from concourse.bass_utils import run_bass_kernel_spmd

    in_maps, dts, use_bias = _prep(inputs)
    key = (dts, use_bias)
    if key not in _cache:
        _cache[key] = _build(dts, use_bias)
    nc = _cache[key]

    res = run_bass_kernel_spmd(nc, in_maps, list(range(NCORES)),
                               trace=trace, **(trace_kwargs or {}))
    mu = np.empty((N_TRAJ, LAT), np.float32)
    std = np.empty((N_TRAJ, LAT), np.float32)
    for c in range(NCORES):
        z = res.results[c]["zout"]
        mu[c * B:(c + 1) * B] = z[0:LAT].T
        std[c * B:(c + 1) * B] = z[LAT:2 * LAT].T
    return (mu[None], std[None]), res


def kernel(**inputs):
    out, _ = _run(inputs, trace=False)
    return out
